# revision 1
# baseline (speedup 1.0000x reference)
"""Trainium2 Bass kernel for nn_BilinearEquivariantLayer (v2).

Per core c of 8, SPMD, all-bf16 dataflow (f32 PSUM accumulation):
  stage 1: A_pos[k] = P[k] @ V[k] for k in {2c, 2c+1}      (k-sharded)
           V columns pre-permuted host-side to dest-major strip order:
           core c owns global cols {32c..32c+32} u {256+32c..256+32c+32}.
  A2A    : ONE fused AllToAll (bf16, 2MB) -> core c holds all k for its
           64 owned columns.
  stage 2: irfft over k as matmuls vs CIR -> A_real in SBUF (bf16)
  stage 3: W1A (both 32-col strips) -> DRAM; AG-A (strip A of all ranks
           = global cols 0..256), AG-B (cols 256..512); then W2A -> SBUF.
  stage 4/5 pass X in {A,B}: bilinear U[t,h] = W2A[t,h].T @ W1A_X[t,h]
           (own 64 rows, 256 gathered cols); U staged to SBUF in
           ti-partition layout via SBUF->SBUF flatten DMAs; fused
           rfft+mixer matmul (G) -> bf16 output planes, pass X columns.
  Host assembles complex64 output from bf16 planes.
"""
import sys
sys.path.insert(0, "/opt/trn_rl_repo")
import os
import numpy as np
from concourse import bass, bacc, tile, mybir
from concourse import bass_utils

NCORES = 8
K, D, N, R, H, dproj = 16, 512, 1024, 512, 8, 128
T = 2 * K - 1           # 31
KL = K // NCORES        # 2 k's per core
RC = 64                 # own columns per core (2 strips of 32)
SW = 32                 # strip width
F32 = mybir.dt.float32
BF16 = mybir.dt.bfloat16

_CACHE = {}


def _build():
    nc = bacc.Bacc("TRN2", target_bir_lowering=False, debug=False,
                   num_devices=NCORES)
    pt = nc.dram_tensor("pt", [KL, 2, N, D], BF16, kind="ExternalInput").ap()
    v = nc.dram_tensor("v", [KL, N, R], BF16, kind="ExternalInput").ap()
    w1t = nc.dram_tensor("w1t", [D, H * dproj], BF16,
                         kind="ExternalInput").ap()
    w2t = nc.dram_tensor("w2t", [D, H * dproj], BF16,
                         kind="ExternalInput").ap()
    # block-diag irfft matrix: cirb[(r4 p), (r4' t)] = cir[p, t] * (r4==r4')
    cirb = nc.dram_tensor("cirb", [128, 128], BF16, kind="ExternalInput").ap()
    g = nc.dram_tensor("g", [2, 128, 256], BF16, kind="ExternalInput").ap()
    # out planes: [m(re/im), pass, (k j), (sc, s')]
    o = nc.dram_tensor("o", [2, 2, 128, 64 * 256], BF16,
                       kind="ExternalOutput").ap()

    with tile.TileContext(nc) as tc:
        with tc.tile_pool(name="dram", bufs=1, space="DRAM") as dram:
            # per-kl A2A buffers: [dest, rc, ri, D] (A_pos transposed)
            a2a_in = [dram.tile([NCORES, RC, 2, D], BF16, name=f"a2ain{kl}")
                      for kl in range(KL)]
            a2a_out = [dram.tile([NCORES, RC, 2, D], BF16,
                                 name=f"a2aout{kl}") for kl in range(KL)]
            w1a_loc = [dram.tile([H, dproj, 32, SW], BF16,
                                 name=f"w1aloc{x}") for x in range(2)]
            w1a_ag = [dram.tile([NCORES, H, dproj, 32, SW], BF16,
                                addr_space="Shared", name=f"w1aag{x}")
                      for x in range(2)]
            # u bounce buffers, sc-major: [sc 64, tq 8, i 4, 256]
            u_dr = [[[dram.tile([RC, 8, 4, 256], BF16,
                                name=f"udr{x}{c}{h4}") for h4 in range(4)]
                     for c in range(2)] for x in range(2)]

            with tc.tile_pool(name="big", bufs=1) as big:
                # tensors living through the tail
                w2a_sb = big.tile([dproj, H, 32, 2, SW], BF16)
                g_sb = big.tile([128, 2, 256], BF16)
                nc.scalar.dma_start(out=g_sb[:, 0, :], in_=g[0])
                nc.scalar.dma_start(out=g_sb[:, 1, :], in_=g[1])

                # ---- stage 1: A_pos = P @ V (own k's) -> a2a_in
                sc1 = nc.named_scope("st1"); sc1.__enter__()
                with tc.tile_pool(name="s1", bufs=1) as s1, \
                     tc.tile_pool(name="s1c", bufs=4) as s1c, \
                     tc.tile_pool(name="ps1", bufs=3, space="PSUM") as ps1p:
                    pt_sb = s1.tile([128, KL, 2, 8, D], BF16)
                    v_sb = s1.tile([128, KL, 8, R], BF16)
                    for kl in range(KL):
                        for nc0 in range(0, 8, 2):
                            nc.sync.dma_start(
                                out=v_sb[:, kl, nc0:nc0 + 2, :],
                                in_=v[kl, nc0 * 128:(nc0 + 2) * 128,
                                      :].rearrange("(a p) d -> p a d", p=128))
                            for ri in range(2):
                                eng = nc.scalar if ri == 0 else nc.sync
                                eng.dma_start(
                                    out=pt_sb[:, kl, ri, nc0:nc0 + 2, :],
                                    in_=pt[kl, ri, nc0 * 128:(nc0 + 2) * 128,
                                           :].rearrange("(a p) d -> p a d",
                                                        p=128))
                    for kl in range(KL):
                        for ri in range(2):
                            for rcc in range(4):
                                # psum = A_pos^T chunk [rc 128, D 512]
                                ps1 = ps1p.tile([128, D], F32, tag="ps1")
                                for nci in range(8):
                                    nc.tensor.matmul(
                                        ps1[:],
                                        v_sb[:, kl, nci,
                                             rcc * 128:(rcc + 1) * 128],
                                        pt_sb[:, kl, ri, nci, :],
                                        start=(nci == 0), stop=(nci == 7))
                                cp1 = s1c.tile([128, D], BF16, tag="cp1")
                                nc.vector.tensor_copy(cp1[:], ps1[:])
                                for i in range(2):
                                    eng = nc.sync if i == 0 else nc.scalar
                                    eng.dma_start(
                                        out=a2a_in[kl][rcc * 2 + i,
                                                       :, ri, :],
                                        in_=cp1[i * 64:(i + 1) * 64, :])
                        nc.gpsimd.collective_compute(
                            "AllToAll", mybir.AluOpType.bypass,
                            replica_groups=[list(range(NCORES))],
                            ins=[a2a_in[kl].opt()],
                            outs=[a2a_out[kl].opt()])
                sc1.__exit__(None, None, None)

                # ---- stages 2+3
                sc2 = nc.named_scope("st23"); sc2.__enter__()
                with tc.tile_pool(name="mid", bufs=1) as mid, \
                     tc.tile_pool(name="a2asb", bufs=2) as a2ap, \
                     tc.tile_pool(name="wcst", bufs=6) as wcst, \
                     tc.tile_pool(name="ps2", bufs=4, space="PSUM") as ps2p, \
                     tc.tile_pool(name="ps3", bufs=4, space="PSUM") as ps3p:
                    # ar free layout: [dc, strip, t(32), rc(32)]
                    ar_sb = mid.tile([128, 4, 2, 32, SW], BF16)
                    w1t_sb = mid.tile([128, 4, H * dproj], BF16)
                    w2t_sb = mid.tile([128, 4, H * dproj], BF16)
                    cirb_sb = mid.tile([128, 128], BF16)
                    nc.scalar.dma_start(out=cirb_sb[:], in_=cirb[:, :])
                    for dc in range(4):
                        nc.scalar.dma_start(
                            out=w1t_sb[:, dc, :],
                            in_=w1t[dc * 128:(dc + 1) * 128, :])
                        nc.scalar.dma_start(
                            out=w2t_sb[:, dc, :],
                            in_=w2t[dc * 128:(dc + 1) * 128, :])

                    # stage 2: irfft, 4 rc fused into the contraction.
                    # a2aq partitions = (kl, src, r4, ri); free = D.
                    def st2_half(half):
                        ps2l = {}
                        for rq in range(half * 8, half * 8 + 8):
                            a2aq = a2ap.tile([128, D], BF16, tag="a2aq",
                                             name="a2aq")
                            for kl in range(KL):
                                eng = nc.sync if kl == 0 else nc.scalar
                                eng.dma_start(
                                    out=a2aq[kl * 64:(kl + 1) * 64, :],
                                    in_=a2a_out[kl][
                                        :, rq * 4:(rq + 1) * 4, :,
                                        :].rearrange("s r k d -> s (r k) d"))
                            rqg, j = rq // 4, rq % 4
                            for dc in range(4):
                                if j == 0:
                                    ps2l[dc] = ps2p.tile([128, 512], F32,
                                                         tag="ps2",
                                                         name=f"ps2_{dc}")
                                nc.tensor.matmul(
                                    ps2l[dc][:, j * 128:(j + 1) * 128],
                                    a2aq[:, dc * 128:(dc + 1) * 128],
                                    cirb_sb[:],
                                    start=True, stop=True)
                            if j == 3:
                                strip, off = rqg // 2, (rqg % 2) * 16
                                for dc in range(4):
                                    nc.vector.tensor_copy(
                                        ar_sb[:, dc, strip, :, off:off + 16],
                                        ps2l[dc][:].rearrange(
                                            "p (j r t) -> p t (j r)",
                                            j=4, r=4))

                    # stage 3: one W-projection pass for a column strip
                    def st3_W(wsb, strip, wi):
                        for hp in range(4):
                            pss = [ps3p.tile([128, 512], F32, tag="ps3",
                                             name=f"ps3_{q}")
                                   for q in range(4)]
                            for dc in range(4):
                                for hh in range(2):
                                    h = hp * 2 + hh
                                    for th in range(2):
                                        nc.tensor.matmul(
                                            pss[hh * 2 + th][:],
                                            wsb[:, dc,
                                                h * 128:(h + 1) * 128],
                                            ar_sb[:, dc, strip,
                                                  th * 16:(th + 1) * 16, :],
                                            start=(dc == 0),
                                            stop=(dc == 3))
                            for hh in range(2):
                                h = hp * 2 + hh
                                for th in range(2):
                                    ps = pss[hh * 2 + th]
                                    if wi == 0:
                                        wc = wcst.tile([128, 512], BF16,
                                                       tag="wc")
                                        nc.vector.tensor_copy(wc[:], ps[:])
                                        weng = (nc.sync if (hh + th) % 2 == 0
                                                else nc.scalar)
                                        weng.dma_start(
                                            out=w1a_loc[strip][
                                                h, :,
                                                th * 16:(th + 1) * 16, :],
                                            in_=wc[:].rearrange(
                                                "p (t r) -> p t r", t=16))
                                    elif th == 0:
                                        nc.vector.tensor_copy(
                                            w2a_sb[:, h,
                                                   th * 16:(th + 1) * 16,
                                                   strip, :],
                                            ps[:].rearrange(
                                                "p (t r) -> p t r", t=16))
                                    else:
                                        nc.scalar.copy(
                                            w2a_sb[:, h,
                                                   th * 16:(th + 1) * 16,
                                                   strip, :],
                                            ps[:].rearrange(
                                                "p (t r) -> p t r", t=16))

                    for strip in range(2):
                        st2_half(strip)
                        st3_W(w1t_sb, strip, 0)
                        nc.gpsimd.collective_compute(
                            "AllGather", mybir.AluOpType.bypass,
                            replica_groups=[list(range(NCORES))],
                            ins=[w1a_loc[strip].opt()],
                            outs=[w1a_ag[strip].opt()])
                    for strip in range(2):
                        st3_W(w2t_sb, strip, 1)
                sc2.__exit__(None, None, None)

                # ---- stages 4+5, pass X over gathered column halves
                sc4 = nc.named_scope("st45"); sc4.__enter__()
                with tc.tile_pool(name="ubig", bufs=1) as ubig, \
                     tc.tile_pool(name="w1x", bufs=6) as w1xp, \
                     tc.tile_pool(name="ust", bufs=5) as ustp, \
                     tc.tile_pool(name="ost", bufs=2) as ostp, \
                     tc.tile_pool(name="ps4", bufs=2, space="PSUM") as ps4p, \
                     tc.tile_pool(name="ps5", bufs=4, space="PSUM") as ps5p:
                    u_sb = [[ubig.tile([128, RC * 256], BF16,
                                       name=f"u{x}{c}") for c in range(2)]
                            for x in range(2)]

                    def w1x_load(X, s, engs2):
                        # s = flat (hp, t8) index; two tiles [d, rb, 8t, rc]
                        # each tile split across both queues (rb halves)
                        hp, t8 = s // 4, s % 4
                        pair = []
                        for hh in range(2):
                            w1xt = w1xp.tile([dproj, NCORES, 8, SW], BF16,
                                             tag="w1x", name="w1x")
                            for rh in range(2):
                                engs2[rh].dma_start(
                                    out=w1xt[:, rh * 4:(rh + 1) * 4, :, :],
                                    in_=w1a_ag[X][rh * 4:(rh + 1) * 4,
                                                  2 * hp + hh, :,
                                                  t8 * 8:(t8 + 1) * 8,
                                                  :].transpose([1, 0, 2, 3]))
                            pair.append(w1xt)
                        return pair

                    def stage4(X):
                        # head-pair packed: psum rows 0:64 = head 2hp,
                        # rows 64:128 = head 2hp+1
                        engs2 = ((nc.sync, nc.scalar) if X == 0
                                 else (nc.gpsimd, nc.scalar))
                        # prefetch two s-steps ahead to ride out
                        # contended-load latency under the AllGather
                        pending = [w1x_load(X, 0, engs2),
                                   w1x_load(X, 1, engs2)]
                        for s in range(16):
                            cur = pending.pop(0)
                            if s + 2 < 16:
                                pending.append(w1x_load(X, s + 2, engs2))
                            hp, t8 = s // 4, s % 4
                            cc = hp // 2
                            for tqq in range(2):
                                tq = t8 * 2 + tqq
                                t0 = 4 * tq
                                ps4 = ps4p.tile([128, 1024], F32, tag="ps4")
                                for i in range(4):
                                    tloc = t0 + i - t8 * 8
                                    for hh in range(2):
                                        nc.tensor.matmul(
                                            ps4[hh * 64:(hh + 1) * 64,
                                                i * 256:(i + 1) * 256],
                                            w2a_sb[:, 2 * hp + hh,
                                                   t0 + i, :, :],
                                            cur[hh][:, :, tloc, :],
                                            start=True, stop=True)
                                ust = ustp.tile([128, 1024], BF16,
                                                tag="ust")
                                if tq % 2 == 0:
                                    nc.vector.tensor_copy(ust[:], ps4[:])
                                else:
                                    nc.scalar.copy(ust[:], ps4[:])
                                for hh in range(2):
                                    h4 = (2 * hp + hh) % 4
                                    eng = nc.sync if hh == 0 else nc.scalar
                                    eng.dma_start(
                                        out=u_dr[X][cc][h4][:, tq, :, :],
                                        in_=ust[hh * 64:(hh + 1) * 64,
                                                :].rearrange(
                                            "p (i f) -> p i f", i=4))
                        # readbacks: pass A on sync/scalar (keeps gpsimd
                        # clear for pass-B loads); pass B on gpsimd
                        for cc in range(2):
                            for h4 in range(4):
                                if X == 0:
                                    eng = nc.sync if h4 % 2 == 0 else nc.scalar
                                else:
                                    eng = nc.gpsimd
                                eng.dma_start(
                                    out=u_sb[X][cc][
                                        h4 * 32:(h4 + 1) * 32, :].rearrange(
                                        "p (sc f) -> p sc f", sc=64),
                                    in_=u_dr[X][cc][h4].transpose(
                                        [1, 2, 0, 3]).rearrange(
                                        "tq i sc f -> (tq i) sc f"))

                    def stage5(X):
                        for fcg in range(8):
                            for mc in range(2):
                                ost = ostp.tile([128, 2048], BF16, tag="ost")
                                for f in range(4):
                                    fc = fcg * 4 + f
                                    ps5 = ps5p.tile([128, 512], F32,
                                                    tag="ps5")
                                    for cci in range(2):
                                        nc.tensor.matmul(
                                            ps5[:],
                                            g_sb[:, cci,
                                                 mc * 128:(mc + 1) * 128],
                                            u_sb[X][cci][
                                                :, fc * 512:(fc + 1) * 512],
                                            start=(cci == 0), stop=(cci == 1))
                                    nc.vector.tensor_copy(
                                        ost[:, f * 512:(f + 1) * 512],
                                        ps5[:])
                                oeng = nc.sync if mc == 0 else nc.scalar
                                oeng.dma_start(
                                    out=o[mc, X, :,
                                          fcg * 2048:(fcg + 1) * 2048],
                                    in_=ost[:])

                    stage4(0)
                    stage4(1)
                    stage5(0)
                    stage5(1)
                sc4.__exit__(None, None, None)
    nc.compile()
    return nc


def _host_prep(P_real, P_imag, V, W1, W2, mixer_real, mixer_imag):
    from ml_dtypes import bfloat16 as bf16
    P_real = np.asarray(P_real, np.float32)
    P_imag = np.asarray(P_imag, np.float32)
    V = np.asarray(V, np.float32)
    W1 = np.asarray(W1, np.float32)
    W2 = np.asarray(W2, np.float32)
    mr = np.asarray(mixer_real, np.float32)
    mi = np.asarray(mixer_imag, np.float32)

    # (K, 2, N, D) bf16
    pt_all = np.stack([P_real.transpose(0, 2, 1),
                       P_imag.transpose(0, 2, 1)], axis=1).astype(bf16)
    # V columns permuted dest-major: core c owns {32c..32c+32, 256+32c..}
    perm = np.concatenate(
        [np.r_[32 * c:32 * c + 32, 256 + 32 * c:256 + 32 * c + 32]
         for c in range(NCORES)])
    v_all = np.ascontiguousarray(V[:, :, perm]).astype(bf16)
    w1t = np.ascontiguousarray(W1.reshape(H * dproj, D).T).astype(bf16)
    w2t = np.ascontiguousarray(W2.reshape(H * dproj, D).T).astype(bf16)

    t_idx, k_idx = np.arange(T), np.arange(K)
    ang = 2 * np.pi * np.outer(k_idx, t_idx) / T
    scale = np.where(k_idx[:, None] == 0, 1.0, 2.0) / T
    cr_k = np.cos(ang) * scale
    ci_k = -np.sin(ang) * scale
    # contraction row order p = kl*64 + src*8 + r4*2 + ri, k = 2*src + kl;
    # block-diag over the 4 fused rc columns (col blocks r4'*32 + t)
    cirbm = np.zeros((128, 128), np.float32)
    for p in range(128):
        kl_, rem = p // 64, p % 64
        src, r4, ri_ = rem // 8, (rem % 8) // 2, rem % 2
        k_ = 2 * src + kl_
        coef = cr_k[k_] if ri_ == 0 else ci_k[k_]
        cirbm[p, r4 * 32:r4 * 32 + T] = coef
    cirbm = cirbm.astype(bf16)

    cos2, sin2 = np.cos(ang), np.sin(ang)  # (K, T)
    G = np.empty((H, T, 2, K, H), np.float32)
    G[:, :, 0] = (np.einsum('kt,ij->itkj', cos2, mr)
                  + np.einsum('kt,ij->itkj', sin2, mi))
    G[:, :, 1] = (np.einsum('kt,ij->itkj', cos2, mi)
                  - np.einsum('kt,ij->itkj', sin2, mr))
    gm124 = G.reshape(2, 124, 256)
    # pad t axis to 32 per h%4 group (row p0 = (h%4)*32 + t, t=31 row zero)
    gm = np.zeros((2, 128, 256), np.float32)
    for h4 in range(4):
        gm[:, h4 * 32:h4 * 32 + 31, :] = gm124[:, h4 * 31:(h4 + 1) * 31, :]
    gm = np.ascontiguousarray(gm).astype(bf16)

    in_maps = []
    for c in range(NCORES):
        in_maps.append({
            "pt": np.ascontiguousarray(pt_all[2 * c:2 * c + 2]),
            "v": np.ascontiguousarray(v_all[2 * c:2 * c + 2]),
            "w1t": w1t, "w2t": w2t, "cirb": cirbm, "g": gm,
        })
    return in_maps


def _assemble(outs):
    res = np.empty((K, R, R, H), np.complex64)
    # res axes viewed (k, passR, rbR, rcpR | stripS, cS, iS | j)
    rv = res.reshape(K, 2, 8, 32, 2, 8, 32, H)
    for c in range(NCORES):
        oc = np.asarray(outs[c], dtype=np.float32)  # (2, 2, 128, 16384)
        arr = (oc[0] + 1j * oc[1]).astype(np.complex64)
        # (pass, k, j, strip, i, rb, rcp)
        arr = arr.reshape(2, K, H, 2, 32, 8, 32)
        rv[:, :, :, :, :, c, :, :] = arr.transpose(1, 0, 5, 6, 3, 4, 2)
    return res


def _enable_axon_trace():
    """Dev-only: register the NTFF profile hook (missing antenv.axon_hooks)
    and stub the artifact upload so run_bass_kernel_spmd(trace=True) works."""
    import types
    if "antenv.axon_hooks" not in sys.modules:
        m = types.ModuleType("antenv.axon_hooks")
        m._hook = None
        m.set_axon_ntff_profile_hook = lambda h: setattr(m, "_hook", h)
        m.get_axon_ntff_profile_hook = lambda: m._hook
        sys.modules["antenv.axon_hooks"] = m
        import antenv
        antenv.axon_hooks = m
        from trn_agent_boot.trn_boot import _ntff_profile_via_ctypes
        hook = _ntff_profile_via_ctypes("/opt/axon/libaxon_pjrt.so")
        m._hook = hook
    bass_utils.upload_artifacts = lambda tmpdir: f"local:{tmpdir}"


def kernel(P_real, P_imag, V, W1, W2, mixer_real, mixer_imag):
    if "nc" not in _CACHE:
        _CACHE["nc"] = _build()
    nc = _CACHE["nc"]
    in_maps = _host_prep(P_real, P_imag, V, W1, W2, mixer_real, mixer_imag)

    if os.environ.get("KSIM"):
        from concourse.bass_interp import MultiCoreSim
        sim = MultiCoreSim(nc, num_cores=NCORES, num_workers=NCORES)
        for c in range(NCORES):
            for k_, arr in in_maps[c].items():
                sim.cores[c].tensor(k_)[:] = arr
        sim.simulate(check_with_hw=False)
        outs = [np.array(sim.cores[c].tensor("o")) for c in range(NCORES)]
        return _assemble(outs)

    trace = bool(os.environ.get("KTRACE"))
    if trace:
        _enable_axon_trace()
    res = bass_utils.run_bass_kernel_spmd(
        nc, in_maps, core_ids=list(range(NCORES)), trace=trace,
        tmpdir=os.environ.get("KTRACE_DIR") or None)
    if trace:
        print(f"HW exec time: {res.exec_time_ns} ns")
        _CACHE["exec_time_ns"] = res.exec_time_ns
        _CACHE["results"] = res
    outs = [res.results[c]["o"] for c in range(NCORES)]
    return _assemble(outs)



# revision 15
# speedup vs baseline: 1.1020x; 1.1020x over previous
"""Trainium2 Bass kernel for nn_BilinearEquivariantLayer (v4, pair-sharded).

Per core c of 8, SPMD. Core c's own columns: cols(c) = {32c..32c+32} u
{256+32c..256+32c+32} (host V permutation). Pair partner p = c^4; group
G = c//4 (cores 0-3 / 4-7).

  st1: A_pos^T[k] = (P[k] @ V[k])^T for k in {2c, 2c+1}; each 32-row
       strip is routed to TWO dests (owner and owner^4); two AllToAlls
       redistribute so core c holds all k for l-set = cols(c)+cols(c^4)
       (128 cols; l = q*64 + blk*32 + sw, q=0 own / q=1 partner).
  st2: irfft over k as matmul vs block-diag CIR -> A_real in SBUF
       [D-chunk p, dc4, t32, l128] (bf16).
  st3: per t8-group tg: W1A[t,h,d,own 64] -> grouped AllGather
       (replica groups [0-3],[4-7], 4 chunks); W2A[t,h,d,l128] -> SBUF.
  st4: bilinear U[t,h][l128, s~256] = W2A^T @ W1A_group per (t, head),
       full m=128 matmuls; s~ = (gi4, c64) = group G's columns. U
       staged to DRAM u_dr[cc, (h4 t), l, s~] via one 512KB DMA per
       (tg, h).
  st5: fused rfft+mixer as matmul G^T @ U, streaming u_dr in 512KB
       slices; output planes o[mc, (k j), (l128, s~256)].
Host assembles complex64 output.
"""
import sys
sys.path.insert(0, "/opt/trn_rl_repo")
import os
import numpy as np
from concourse import bass, bacc, tile, mybir
from concourse import bass_utils

NCORES = 8
K, D, N, R, H, dproj = 16, 512, 1024, 512, 8, 128
T = 2 * K - 1           # 31
KL = K // NCORES        # 2 k's per core
LC = 128                # l-set size (own + partner columns)
F32 = mybir.dt.float32
BF16 = mybir.dt.bfloat16

# precision mode: "bf16" (safe) or "e3m4" (fp8 W1A, halves AllGather bytes)
PREC = os.environ.get("KPREC", "bf16")
if PREC == "e3m4":
    W1A_DT = mybir.dt.float8e3
    W1_SCALE = 2.0 ** -8
    G_SCALE = 2.0 ** 8
else:
    W1A_DT = BF16
    W1_SCALE = 1.0
    G_SCALE = 1.0
W2_SCALE = 1.0
U_DT = BF16

_CACHE = {}


def _copy(nc, i, out, in_):
    """Rotate psum->sbuf copies across vector/scalar (gpsimd can't read
    PSUM)."""
    if i % 2 == 0:
        nc.vector.tensor_copy(out, in_)
    else:
        nc.scalar.copy(out, in_)


def _build():
    nc = bacc.Bacc("TRN2", target_bir_lowering=False, debug=False,
                   num_devices=NCORES)
    pt = nc.dram_tensor("pt", [KL, 2, N, D], BF16, kind="ExternalInput").ap()
    v = nc.dram_tensor("v", [KL, N, R], BF16, kind="ExternalInput").ap()
    w1t = nc.dram_tensor("w1t", [D, H * dproj], BF16,
                         kind="ExternalInput").ap()
    w2t = nc.dram_tensor("w2t", [D, H * dproj], BF16,
                         kind="ExternalInput").ap()
    cirb = nc.dram_tensor("cirb", [128, 128], BF16, kind="ExternalInput").ap()
    g = nc.dram_tensor("g", [2, 128, 256], BF16, kind="ExternalInput").ap()
    # out planes: [m(re/im), (k j), (l128, s~256)]
    o = nc.dram_tensor("o", [2, 128, LC * 256], BF16,
                       kind="ExternalOutput").ap()

    with tile.TileContext(nc) as tc:
        with tc.tile_pool(name="dram", bufs=1, space="DRAM") as dram:
            # A2A buffers: [kl, dest, l-slot 128, ri, D]
            a2a_in = dram.tile([KL, NCORES, LC, 2, D], BF16, name="a2ain")
            a2a_out = dram.tile([KL, NCORES, LC, 2, D], BF16, name="a2aout")
            # W1A grouped AllGather, 4 t8-chunks: [h, d, t8, c64]
            ag_in = [dram.tile([H, dproj, 8, 64], W1A_DT, name=f"agin{tg}")
                     for tg in range(4)]
            w1ag = [dram.tile([4, H, dproj, 8, 64], W1A_DT,
                              name=f"w1ag{tg}")
                    for tg in range(4)]
            # U staging: [cc, (h4 t)=128, l128, s~256]
            u_dr = dram.tile([2, 128, LC, 256], U_DT, name="udr")

            with tc.tile_pool(name="big", bufs=1) as big:
                w2a_sb = big.tile([dproj, H, 32, LC], BF16)   # 8.4MB
                g_sb = big.tile([128, 2, 256], BF16)
                nc.scalar.dma_start(out=g_sb[:], in_=g.rearrange(
                    "a p b -> p a b"))

                # ---- stage 1: A_pos^T -> a2a_in (strips to 2 dests)
                sc1 = nc.named_scope("st1"); sc1.__enter__()
                with tc.tile_pool(name="s1", bufs=1) as s1, \
                     tc.tile_pool(name="s1c", bufs=4) as s1c, \
                     tc.tile_pool(name="ps1", bufs=3, space="PSUM") as ps1p:
                    pt_sb = s1.tile([128, KL, 2, 8, D], BF16)
                    v_sb = s1.tile([128, KL, 8, R], BF16)
                    for kl in range(KL):
                        eng = nc.sync if kl == 0 else nc.scalar
                        eng.dma_start(
                            out=v_sb[:, kl, :, :],
                            in_=v[kl].rearrange("(a p) d -> p a d", p=128))
                        for ri in range(2):
                            eng = nc.sync if ri == 0 else nc.scalar
                            eng.dma_start(
                                out=pt_sb[:, kl, ri, :, :],
                                in_=pt[kl, ri].rearrange(
                                    "(a p) d -> p a d", p=128))
                    for kl in range(KL):
                        for ri in range(2):
                            for rcc in range(4):
                                # psum = A_pos^T chunk [rc 128, D 512]
                                ps1 = ps1p.tile([128, D], F32, tag="ps1")
                                for nci in range(8):
                                    nc.tensor.matmul(
                                        ps1[:],
                                        v_sb[:, kl, nci,
                                             rcc * 128:(rcc + 1) * 128],
                                        pt_sb[:, kl, ri, nci, :],
                                        start=(nci == 0), stop=(nci == 7))
                                cp1 = s1c.tile([128, D], BF16, tag="cp1")
                                _copy(nc, rcc, cp1[:], ps1[:])
                                # rows = cores {2rcc, 2rcc+1} x 64; send to
                                # owner (q=0 slots) and partner (q=1 slots)
                                qd = (2 * rcc + 4) % 8
                                nc.gpsimd.dma_start(
                                    out=a2a_in[kl, 2 * rcc:2 * rcc + 2,
                                               0:64, ri, :],
                                    in_=cp1[:])
                                deng = nc.sync if rcc % 2 == 0 else nc.scalar
                                deng.dma_start(
                                    out=a2a_in[kl, qd:qd + 2,
                                               64:128, ri, :],
                                    in_=cp1[:])
                        nc.gpsimd.collective_compute(
                            "AllToAll", mybir.AluOpType.bypass,
                            replica_groups=[list(range(NCORES))],
                            ins=[a2a_in[kl].opt()],
                            outs=[a2a_out[kl].opt()])
                sc1.__exit__(None, None, None)

                # ---- stage 2: irfft -> ar_sb [128 Dp, dc4, t32, l128]
                sc2 = nc.named_scope("st2"); sc2.__enter__()
                with tc.tile_pool(name="mid", bufs=1) as mid, \
                     tc.tile_pool(name="a2asb", bufs=4) as a2ap, \
                     tc.tile_pool(name="ps2", bufs=4, space="PSUM") as ps2p:
                    ar_sb = mid.tile([128, 4, 32, LC], BF16)
                    w1t_sb = mid.tile([128, 4, H * dproj], BF16)
                    w2t_sb = mid.tile([128, 4, H * dproj], BF16)
                    cirb_sb = mid.tile([128, 128], BF16)
                    nc.scalar.dma_start(out=cirb_sb[:], in_=cirb[:, :])
                    nc.scalar.dma_start(
                        out=w1t_sb[:],
                        in_=w1t.rearrange("(a p) f -> p a f", p=128))
                    nc.scalar.dma_start(
                        out=w2t_sb[:],
                        in_=w2t.rearrange("(a p) f -> p a f", p=128))

                    # a2aq partitions = (kl, src, r4, ri); free = D.
                    ps2l = {}
                    for rq in range(32):
                        a2aq = a2ap.tile([128, D], BF16, tag="a2aq",
                                         name="a2aq")
                        eng = nc.sync if rq % 2 == 0 else nc.scalar
                        eng.dma_start(
                            out=a2aq[:],
                            in_=a2a_out[:, :, rq * 4:(rq + 1) * 4, :,
                                        :].rearrange(
                                "a s r k d -> (a s) (r k) d"))
                        rqg, j = rq // 4, rq % 4
                        for dc in range(4):
                            if j == 0:
                                ps2l[dc] = ps2p.tile([128, 512], F32,
                                                     tag="ps2",
                                                     name=f"ps2_{dc}")
                            nc.tensor.matmul(
                                ps2l[dc][:, j * 128:(j + 1) * 128],
                                a2aq[:, dc * 128:(dc + 1) * 128],
                                cirb_sb[:],
                                start=True, stop=True)
                        if j != 3:
                            continue
                        for dc in range(4):
                            _copy(nc, dc,
                                  ar_sb[:, dc, :,
                                        rqg * 16:rqg * 16 + 16],
                                  ps2l[dc][:].rearrange(
                                      "p (j r t) -> p t (j r)",
                                      j=4, r=4))
                    sc2.__exit__(None, None, None)

                    # ---- stage 3: W-projections per t8-group
                    sc3 = nc.named_scope("st3"); sc3.__enter__()
                    with tc.tile_pool(name="agst", bufs=2) as agstp, \
                         tc.tile_pool(name="ps3", bufs=3, space="PSUM") \
                            as ps3p:
                        def proj(wsb, tg, h, l0):
                            ps3 = ps3p.tile([128, 512], F32, tag="ps3")
                            for dc in range(4):
                                nc.tensor.matmul(
                                    ps3[:],
                                    wsb[:, dc, h * 128:(h + 1) * 128],
                                    ar_sb[:, dc, tg * 8:(tg + 1) * 8,
                                          l0:l0 + 64],
                                    start=(dc == 0), stop=(dc == 3))
                            return ps3

                        for tg in range(4):
                            stg = agstp.tile([128, H, 512], W1A_DT,
                                             tag="stg")
                            for h in range(H):
                                ps3 = proj(w1t_sb, tg, h, 0)
                                _copy(nc, h, stg[:, h, :], ps3[:])
                            # one DMA: [d p, (h,t8,c)] -> [h,d,t8,c]
                            nc.gpsimd.dma_start(
                                out=ag_in[tg].transpose([1, 0, 2, 3]),
                                in_=stg[:])
                            nc.gpsimd.collective_compute(
                                "AllGather", mybir.AluOpType.bypass,
                                replica_groups=[[0, 1, 2, 3],
                                                [4, 5, 6, 7]],
                                ins=[ag_in[tg].opt()],
                                outs=[w1ag[tg].opt()])
                            for h in range(H):
                                for lh in range(2):
                                    ps3 = proj(w2t_sb, tg, h, lh * 64)
                                    _copy(nc, h + lh,
                                          w2a_sb[:, h,
                                                 tg * 8:(tg + 1) * 8,
                                                 lh * 64:(lh + 1) * 64],
                                          ps3[:].rearrange(
                                              "p (t c) -> p t c", t=8))
                    sc3.__exit__(None, None, None)

                # ---- stage 4: bilinear per (tg, head), m=128
                sc4 = nc.named_scope("st4"); sc4.__enter__()
                with tc.tile_pool(name="w1x", bufs=6) as w1xp, \
                     tc.tile_pool(name="ust", bufs=3) as ustp, \
                     tc.tile_pool(name="ps4", bufs=3, space="PSUM") as ps4p:
                    def w1x_load(s):
                        tg, h = s // 8, s % 8
                        w1xt = w1xp.tile([dproj, 4, 8, 64], W1A_DT,
                                         tag="w1x", name="w1x")
                        eng = nc.sync if s % 2 == 0 else nc.scalar
                        # [d p, (gi4, t8, c64)] <- w1ag[tg][gi, h]
                        eng.dma_start(
                            out=w1xt[:],
                            in_=w1ag[tg][:, h].transpose([1, 0, 2, 3]))
                        return w1xt

                    pending = [w1x_load(0), w1x_load(1), w1x_load(2)]
                    for s in range(32):
                        tg, h = s // 8, s % 8
                        cc, h4 = h // 4, h % 4
                        cur = pending.pop(0)
                        if s + 3 < 32:
                            pending.append(w1x_load(s + 3))
                        ust = ustp.tile([128, 8, 256], U_DT, tag="ust")
                        for tp in range(4):
                            ps4 = ps4p.tile([128, 512], F32, tag="ps4")
                            for ti in range(2):
                                tl = tp * 2 + ti
                                t = tg * 8 + tl
                                nc.tensor.matmul(
                                    ps4[:, ti * 256:(ti + 1) * 256],
                                    w2a_sb[:, h, t, :],
                                    cur[:, :, tl, :],
                                    start=True, stop=True)
                            _copy(nc, tp,
                                  ust[:, tp * 2:tp * 2 + 2, :],
                                  ps4[:].rearrange("p (a b) -> p a b",
                                                   a=2))
                        rb = h4 * 32 + tg * 8
                        deng = nc.sync if s % 2 == 0 else nc.scalar
                        deng.dma_start(
                            out=u_dr[cc, rb:rb + 8, :, :].transpose(
                                [1, 0, 2]),
                            in_=ust[:])
                sc4.__exit__(None, None, None)

                # ---- stage 5: fused rfft + mixer, streaming u slices
                sc5 = nc.named_scope("st5"); sc5.__enter__()
                with tc.tile_pool(name="u5", bufs=6) as u5p, \
                     tc.tile_pool(name="ost", bufs=3) as ostp, \
                     tc.tile_pool(name="ps5", bufs=4, space="PSUM") as ps5p:
                    def u_load(fcg):
                        pair = []
                        for cci in range(2):
                            ut = u5p.tile([128, 8, 256], U_DT, tag="u5",
                                          name="u5")
                            eng = nc.sync if cci == 0 else nc.scalar
                            eng.dma_start(
                                out=ut[:],
                                in_=u_dr[cci, :, fcg * 8:(fcg + 1) * 8,
                                         :])
                            pair.append(ut)
                        return pair

                    upend = [u_load(0), u_load(1), u_load(2)]
                    for fcg in range(16):
                        uc = upend.pop(0)
                        if fcg + 3 < 16:
                            upend.append(u_load(fcg + 3))
                        for mc in range(2):
                            ost = ostp.tile([128, 2048], BF16, tag="ost")
                            for f in range(4):
                                ps5 = ps5p.tile([128, 512], F32,
                                                tag="ps5")
                                for cci in range(2):
                                    nc.tensor.matmul(
                                        ps5[:],
                                        g_sb[:, cci,
                                             mc * 128:(mc + 1) * 128],
                                        uc[cci][:, f * 2:f * 2 + 2, :],
                                        start=(cci == 0),
                                        stop=(cci == 1))
                                _copy(nc, f,
                                      ost[:, f * 512:(f + 1) * 512],
                                      ps5[:])
                            oeng = nc.sync if mc == 0 else nc.scalar
                            oeng.dma_start(
                                out=o[mc, :,
                                      fcg * 2048:(fcg + 1) * 2048],
                                in_=ost[:])
                sc5.__exit__(None, None, None)
    nc.compile()
    return nc


def _host_prep(P_real, P_imag, V, W1, W2, mixer_real, mixer_imag):
    from ml_dtypes import bfloat16 as bf16
    P_real = np.asarray(P_real, np.float32)
    P_imag = np.asarray(P_imag, np.float32)
    V = np.asarray(V, np.float32)
    W1 = np.asarray(W1, np.float32) * W1_SCALE
    W2 = np.asarray(W2, np.float32) * W2_SCALE
    mr = np.asarray(mixer_real, np.float32)
    mi = np.asarray(mixer_imag, np.float32)

    pt_all = np.stack([P_real.transpose(0, 2, 1),
                       P_imag.transpose(0, 2, 1)], axis=1).astype(bf16)
    perm = np.concatenate(
        [np.r_[32 * c:32 * c + 32, 256 + 32 * c:256 + 32 * c + 32]
         for c in range(NCORES)])
    v_all = np.ascontiguousarray(V[:, :, perm]).astype(bf16)
    w1t = np.ascontiguousarray(W1.reshape(H * dproj, D).T).astype(bf16)
    w2t = np.ascontiguousarray(W2.reshape(H * dproj, D).T).astype(bf16)

    t_idx, k_idx = np.arange(T), np.arange(K)
    ang = 2 * np.pi * np.outer(k_idx, t_idx) / T
    scale = np.where(k_idx[:, None] == 0, 1.0, 2.0) / T
    cr_k = np.cos(ang) * scale
    ci_k = -np.sin(ang) * scale
    cirbm = np.zeros((128, 128), np.float32)
    for p in range(128):
        kl_, rem = p // 64, p % 64
        src, r4, ri_ = rem // 8, (rem % 8) // 2, rem % 2
        k_ = 2 * src + kl_
        coef = cr_k[k_] if ri_ == 0 else ci_k[k_]
        cirbm[p, r4 * 32:r4 * 32 + T] = coef
    cirbm = cirbm.astype(bf16)

    cos2, sin2 = np.cos(ang), np.sin(ang)  # (K, T)
    G = np.empty((H, T, 2, K, H), np.float32)
    G[:, :, 0] = (np.einsum('kt,ij->itkj', cos2, mr)
                  + np.einsum('kt,ij->itkj', sin2, mi))
    G[:, :, 1] = (np.einsum('kt,ij->itkj', cos2, mi)
                  - np.einsum('kt,ij->itkj', sin2, mr))
    G *= G_SCALE
    gm124 = G.reshape(2, 124, 256)
    gm = np.zeros((2, 128, 256), np.float32)
    for h4 in range(4):
        gm[:, h4 * 32:h4 * 32 + 31, :] = gm124[:, h4 * 31:(h4 + 1) * 31, :]
    gm = np.ascontiguousarray(gm).astype(bf16)

    in_maps = []
    for c in range(NCORES):
        in_maps.append({
            "pt": np.ascontiguousarray(pt_all[2 * c:2 * c + 2]),
            "v": np.ascontiguousarray(v_all[2 * c:2 * c + 2]),
            "w1t": w1t, "w2t": w2t, "cirb": cirbm, "g": gm,
        })
    return in_maps


def _assemble(outs):
    res = np.empty((K, R, R, H), np.complex64)
    # rv axes: (k, stripR, cR, swR, stripS, cS, swS, j);
    # R axis (first) <- s~ = (gi, strip', sw'); S axis <- l = (q, blk, sw)
    rv = res.reshape(K, 2, 8, 32, 2, 8, 32, H)
    for c in range(NCORES):
        oc = np.asarray(outs[c], dtype=np.float32)  # (2, 128, 32768)
        arr = (oc[0] + 1j * oc[1]).astype(np.complex64)
        # (k, j, q, blk, sw, gi, strip', sw')
        arr = arr.reshape(K, H, 2, 2, 32, 4, 2, 32)
        gbase = 4 * (c // 4)
        for q in range(2):
            sc = c if q == 0 else c ^ 4
            rv[:, :, gbase:gbase + 4, :, :, sc, :, :] = \
                arr[:, :, q].transpose(0, 5, 4, 6, 2, 3, 1)
    return res


def _enable_axon_trace():
    import types
    if "antenv.axon_hooks" not in sys.modules:
        m = types.ModuleType("antenv.axon_hooks")
        m._hook = None
        m.set_axon_ntff_profile_hook = lambda h: setattr(m, "_hook", h)
        m.get_axon_ntff_profile_hook = lambda: m._hook
        sys.modules["antenv.axon_hooks"] = m
        import antenv
        antenv.axon_hooks = m
        from trn_agent_boot.trn_boot import _ntff_profile_via_ctypes
        hook = _ntff_profile_via_ctypes("/opt/axon/libaxon_pjrt.so")
        m._hook = hook
    bass_utils.upload_artifacts = lambda tmpdir: f"local:{tmpdir}"


def kernel(P_real, P_imag, V, W1, W2, mixer_real, mixer_imag):
    if "nc" not in _CACHE:
        _CACHE["nc"] = _build()
    nc = _CACHE["nc"]
    in_maps = _host_prep(P_real, P_imag, V, W1, W2, mixer_real, mixer_imag)

    if os.environ.get("KSIM"):
        from concourse.bass_interp import MultiCoreSim
        sim = MultiCoreSim(nc, num_cores=NCORES, num_workers=NCORES)
        for c in range(NCORES):
            for k_, arr in in_maps[c].items():
                sim.cores[c].tensor(k_)[:] = arr
        sim.simulate(check_with_hw=False)
        outs = [np.array(sim.cores[c].tensor("o")) for c in range(NCORES)]
        return _assemble(outs)

    trace = bool(os.environ.get("KTRACE"))
    if trace:
        _enable_axon_trace()
    res = bass_utils.run_bass_kernel_spmd(
        nc, in_maps, core_ids=list(range(NCORES)), trace=trace,
        tmpdir=os.environ.get("KTRACE_DIR") or None)
    if trace:
        print(f"HW exec time: {res.exec_time_ns} ns")
        _CACHE["exec_time_ns"] = res.exec_time_ns
        _CACHE["results"] = res
    outs = [res.results[c]["o"] for c in range(NCORES)]
    return _assemble(outs)
